# revision 1
# baseline (speedup 1.0000x reference)
"""Trainium2 Bass kernel for nn_AutoLSTM: conv1d x3 -> LSTM x2 -> dense+BN -> softmax.

Data-parallel over batch: 8 cores x 32 rows, weights replicated.
Layout: free dim is (t, b) time-major with b inner (32), padded by 2 time
steps of zeros each side for the SAME convs.  LSTM runs transposed:
[H=128 partitions, 32 batch] tiles.

Schedule: the 512-step recurrence is latency-bound (~1.2us/step chain:
PE gate matmuls -> fused all-gate sigmoid on ACT -> DVE c-update -> ACT
tanh -> DVE h), so everything else hides inside it:
 - x-projections (zpre) are matmul'd JIT into per-step PSUM banks two
   steps ahead; the recurrent matmuls accumulate on top (only the first
   matmul of a bank may set start=True: start marks the whole 2KB PSUM
   zero-region pending-zero and later writes there OVERWRITE).
 - conv2/conv3 chunks stream into scan1 as a paced thunk queue (~1/step);
   conv activations use Prelu, which shares the ACT function table with
   Sigmoid/Tanh (Lrelu does not -> 1.3us table reload per switch).
 - the im2col for conv2 is built by Pool (GPSIMD) copies; Pool cannot
   access PSUM.
 - dense1 accumulates per-step during scan2 from a DMA ring of Wd1
   chunks; BN stats cross-core reduction is AllGather + local reduce
   (cheaper than AllReduce in latency).

All elementwise scan state is fp16 (DVE 2x mode); rel err ~3.5e-3.

Walrus constraint: engine instructions hold few sync-waits, so
_split_waits hoists extras into standalone EventSemaphore sequencer ops.
"""

import sys

for p in ("/opt/trn_rl_repo",):
    if p not in sys.path:
        sys.path.insert(0, p)

from contextlib import ExitStack

import numpy as np

import concourse.bass as bass
import concourse.mybir as mybir
from concourse.tile import TileContext
from concourse.bass_utils import run_bass_kernel_spmd

F32 = mybir.dt.float32
F16 = mybir.dt.float16
AF = mybir.ActivationFunctionType
ALU = mybir.AluOpType
AX = mybir.AxisListType

NCORES = 8
B = 32          # per-core batch
T = 256
CIN = 8
H = 128
NB = 10
EPS = 1e-5
SLOPE = 0.01    # jax.nn.leaky_relu default

PT = T + 4              # padded time
PF = PT * B             # 8320
F = T * B               # 8192
PAD = 2 * B             # 64
NCH = 16                # 512-wide (16t x 32b) chunks
CH = 512
DCH = 8                 # dense: t-tiles per streamed Wd1 ring chunk
ZQ = 8                  # LSTM steps per PSUM zpre chunk

# on-chip gate order [f, i, o, g]; jax order is [i, f, g, o]
GATE_PERM = [1, 0, 3, 2]


def _h(x):
    return np.asarray(x, dtype=np.float16)


def _f32(x):
    return np.ascontiguousarray(np.asarray(x, dtype=np.float32))


def _perm_gates(w):
    # on-chip order [f, i, o, g]; g-gate preactivation scaled 2x so the
    # fused all-sigmoid gate op yields tanh(zg) = 2*sigmoid(2*zg) - 1
    blocks = [w[..., s * H:(s + 1) * H] for s in GATE_PERM]
    blocks[3] = blocks[3] * 2.0
    return np.concatenate(blocks, axis=-1)


_DBG_RELU = False       # CoreSim lacks Lrelu; swap in Relu for sim debugging
_DBG_DUMP = False       # add intermediate DRAM dumps for debugging


def build_program(lstm_bias=False):
    LSTM_BIAS = lstm_bias
    # Prelu (parametric_relu) == leaky relu with alpha, and it lives in the
    # same activation table set as Sigmoid/Tanh/Identity/Square — no 1.3us
    # table reloads when conv activations interleave with the LSTM scan.
    AF_L = AF.Relu if _DBG_RELU else AF.Prelu
    nc = bass.Bass()

    P = nc.declare_dram_parameter
    xT_d = P("xT", [CIN, PF], F16, isOutput=False)
    w1_d = P("w1", [CIN, 5 * 32], F16, isOutput=False)
    w2a_d = P("w2a", [128, 512], F16, isOutput=False)
    w2b_d = P("w2b", [32, 512], F16, isOutput=False)
    w3_d = P("w3", [128, 20 * 128], F16, isOutput=False)
    wx1_d = P("wx1", [128, 512], F16, isOutput=False)
    wh1_d = P("wh1", [128, 512], F16, isOutput=False)
    wx2_d = P("wx2", [128, 512], F16, isOutput=False)
    wh2_d = P("wh2", [128, 512], F16, isOutput=False)
    b1_d = P("b1c", [128, 4], F32, isOutput=False)
    b2_d = P("b2c", [128, 4], F32, isOutput=False)
    wd1_d = P("wd1", [T * H, 512], F16, isOutput=False)
    bd1_d = P("bd1c", [128, 4], F32, isOutput=False)
    bng_d = P("bng", [128, 4], F32, isOutput=False)
    bnb_d = P("bnb", [128, 4], F32, isOutput=False)
    wd2_d = P("wd2", [128, 4 * NB], F16, isOutput=False)
    bd2_d = P("bd2r", [B, NB], F32, isOutput=False)
    out_d = P("out", [B, NB], F32, isOutput=True)
    if _DBG_DUMP:
        dbg_y3_d = P("dbg_y3", [128, F], F16, isOutput=True)
        dbg_o1_d = P("dbg_o1", [128, F], F16, isOutput=True)
        dbg_o2_d = P("dbg_o2", [128, F], F16, isOutput=True)
        dbg_da_d = P("dbg_da", [128, 4 * B], F32, isOutput=True)

    cc_in = nc.dram_tensor("cc_in", [128, 8], F32)
    cc_out = nc.dram_tensor("cc_out", [NCORES * 128, 8], F32)

    with TileContext(nc) as tc, ExitStack() as ctx:
        mm = lambda *a, **k: nc.tensor.matmul(*a, **k)

        wp = ctx.enter_context(tc.tile_pool(name="wp", bufs=1))
        mp = ctx.enter_context(tc.tile_pool(name="mp", bufs=1))
        pp = ctx.enter_context(tc.tile_pool(name="psum", bufs=1, space="PSUM"))
        wring = ctx.enter_context(tc.tile_pool(name="wring", bufs=2))

        # persistent activation storages
        out1 = mp.tile([128, F], F16)
        out2 = mp.tile([128, F], F16)
        y3 = mp.tile([128, F], F16)
        # scan state ping-pong: cols 0:128 = sigmoid(z) [sf si so sg],
        # 128:160 = c, 160:192 = w = 2*sg-1 (tanh of g-gate preactivation)
        SB0 = mp.tile([128, 192], F16)
        SB1 = mp.tile([128, 192], F16)
        hz = mp.tile([128, B], F16)
        nc.vector.memset(hz, 0.0)
        nc.vector.memset(SB1[:, 128:160], 0.0)

        # conv working buffers: xT dies after conv1; y1/y2 live through
        # scan1 (conv2/conv3 are streamed into it)
        cvpB = tc.tile_pool(name="cvpB", bufs=1)
        cvpA = tc.tile_pool(name="cvpA", bufs=1)
        cpB = cvpB.__enter__()
        cpA = cvpA.__enter__()

        y1 = cpA.tile([32, PF], F16)
        im2 = cpA.tile([128, PF], F16)
        y2 = [cpB.tile([128, PF], F16, name=f"y2_{m}", tag=f"y2_{m}")
              for m in range(4)]

        # x input DMA first — it gates conv1
        cvpX = tc.tile_pool(name="cvpX", bufs=1)
        cpX = cvpX.__enter__()
        xT = cpX.tile([CIN, PF], F16)
        nc.sync.dma_start(out=xT, in_=xT_d[:, :])

        # ---- stage all weights through DVE so matmul operands and ACT
        # bias operands have single-sem producers ----
        with tc.tile_pool(name="stg", bufs=1) as stg:
            def wload(shape, dram, nm, dt=F16, dst=None):
                raw = stg.tile(shape, dt, tag=f"r_{nm}", name=f"r_{nm}")
                nc.sync.dma_start(out=raw, in_=dram[:, :])
                t = dst if dst is not None else wp.tile(shape, dt, name=nm,
                                                        tag=nm)
                nc.vector.tensor_copy(t, raw)
                return t

            w1 = wload([CIN, 5 * 32], w1_d, "w1f")
            w2a = wload([128, 512], w2a_d, "w2af")
            w2b = wload([32, 512], w2b_d, "w2bf")
            w3 = wload([128, 20 * 128], w3_d, "w3f")
            wx1 = wload([128, 512], wx1_d, "wx1f")
            wh1 = wload([128, 512], wh1_d, "wh1f")
            wx2 = wload([128, 512], wx2_d, "wx2f")
            wh2 = wload([128, 512], wh2_d, "wh2f")
            wd2 = wload([128, 4 * NB], wd2_d, "wd2f")
            b1c = wload([128, 4], b1_d, "b1f", F32)
            b2c = wload([128, 4], b2_d, "b2f", F32)
            bd1c = wload([128, 4], bd1_d, "bd1f", F32)
            bng = wload([128, 4], bng_d, "bngf", F32)
            bnb = wload([128, 4], bnb_d, "bnbf", F32)
            bd2r = wload([B, NB], bd2_d, "bd2f", F32)

        # ---------------- conv stack ----------------
        nc.scalar.memzero(y1[:, 0:PAD])
        nc.scalar.memzero(y1[:, PF - PAD:PF])
        for m in range(4):
            nc.scalar.memzero(y2[m][:, 0:PAD])
            nc.scalar.memzero(y2[m][:, PF - PAD:PF])

        # conv1: 5 taps, K=8 -> 32 ch (fully pre-scan)
        for n in range(NCH):
            ps = pp.tile([32, CH], F32, tag="big", bufs=3)
            for k in range(5):
                mm(ps, w1[:, k * 32:(k + 1) * 32],
                   xT[:, n * CH + k * B: n * CH + k * B + CH],
                   start=(k == 0), stop=(k == 4))
            nc.scalar.activation(y1[:, PAD + n * CH: PAD + (n + 1) * CH], ps,
                                 AF_L, alpha=SLOPE)
        cvpX.__exit__(None, None, None)

        # conv2 im2col (taps 0..3) built by Pool copies in PF/4 column
        # quarters — Pool is idle during the scan, and quarter granularity
        # lets early conv2 chunks start before the whole im2col is done.
        nc.gpsimd.memset(im2[0:32, 0:2 * B], 0.0)
        nc.gpsimd.memset(im2[32:64, 0:B], 0.0)
        nc.gpsimd.memset(im2[96:128, PF - B:PF], 0.0)
        QW = PF // 4
        for q in range(4):
            for j in range(4):
                sh = (j - 2) * B
                d0, d1 = q * QW, (q + 1) * QW
                s0, s1 = max(0, d0 + sh), min(PF, d1 + sh)
                if s1 > s0:
                    nc.gpsimd.tensor_copy(
                        im2[j * 32:(j + 1) * 32, s0 - sh:s1 - sh],
                        y1[:, s0:s1])

        # conv2/conv3 are emitted chunk-wise: a prefix pre-scan, the rest
        # drained as small thunks inside scan1 (engines are mostly idle
        # during the latency-bound recurrence).
        def emit_conv2(n):
            for m in range(4):
                ps = pp.tile([128, CH], F32, tag="big", bufs=3)
                mm(ps, w2a[:, m * 128:(m + 1) * 128],
                   im2[:, PAD + n * CH: PAD + (n + 1) * CH],
                   start=True, stop=False)
                mm(ps, w2b[:, m * 128:(m + 1) * 128],
                   y1[:, PAD + n * CH + 2 * B: PAD + n * CH + 2 * B + CH],
                   start=False, stop=True)
                nc.scalar.activation(
                    y2[m][:, PAD + n * CH: PAD + (n + 1) * CH],
                    ps, AF_L, alpha=SLOPE)

        def conv3_mms(ps, n, lo, hi):
            for idx in range(lo, hi):
                k, kt = idx // 4, idx % 4
                mm(ps, w3[:, (k * 4 + kt) * 128:(k * 4 + kt + 1) * 128],
                   y2[kt][:, n * CH + k * B: n * CH + k * B + CH],
                   start=(idx == 0), stop=(idx == 19), skip_group_check=True)

        def emit_conv3(n):
            ps = pp.tile([128, CH], F32, tag="big", bufs=3)
            conv3_mms(ps, n, 0, 20)
            nc.scalar.activation(y3[:, n * CH:(n + 1) * CH], ps,
                                 AF_L, alpha=SLOPE)

        conv_q = []

        def q_conv2(n):
            for m in range(4):
                def mms(n=n, m=m):
                    ps = pp.tile([128, CH], F32, tag="big", bufs=3,
                                 name=f"c2_{n}_{m}")
                    conv_q_state[(n, m)] = ps
                    mm(ps, w2a[:, m * 128:(m + 1) * 128],
                       im2[:, PAD + n * CH: PAD + (n + 1) * CH],
                       start=True, stop=False)
                    mm(ps, w2b[:, m * 128:(m + 1) * 128],
                       y1[:, PAD + n * CH + 2 * B:
                          PAD + n * CH + 2 * B + CH],
                       start=False, stop=True)

                def act(n=n, m=m):
                    ps = conv_q_state.pop((n, m))
                    nc.scalar.activation(
                        y2[m][:, PAD + n * CH: PAD + (n + 1) * CH],
                        ps, AF_L, alpha=SLOPE)
                conv_q.append(mms)
                conv_q.append(act)

        def q_conv3(n):
            for lo in range(0, 20, 3):
                def mms(n=n, lo=lo):
                    if lo == 0:
                        conv_q_state[("c3", n)] = pp.tile(
                            [128, CH], F32, tag="big", bufs=3,
                            name=f"c3_{n}")
                    conv3_mms(conv_q_state[("c3", n)], n, lo,
                              min(lo + 3, 20))
                conv_q.append(mms)

            def act(n=n):
                ps = conv_q_state.pop(("c3", n))
                nc.scalar.activation(y3[:, n * CH:(n + 1) * CH], ps,
                                     AF_L, alpha=SLOPE)
            conv_q.append(act)

        conv_q_state = {}

        # pre-scan prefix: enough conv2/conv3 for the scan's first chunks
        # minimum prefix that gates scan step 0 (zpre(0,0/1) needs y3
        # chunk 0 only); conv2(2)/conv3(1) go to the thunk queue head and
        # drain at 2/step over the first scan steps
        for n in range(2):
            emit_conv2(n)
        emit_conv3(0)
        q_conv2(2)
        q_conv3(1)

        # ---------------- LSTM phase ----------
        # x-projections (zpre) are matmul'd straight into PSUM chunks of
        # ZQ steps (ping-pong); the recurrent gate matmuls accumulate on
        # top and the fused sigmoid reads PSUM directly.
        dacc4 = pp.tile([128, 4 * B], F32, name="dacc4", tag="dacc",
                        bufs=1)
        wd1_tiles = {}
        zs_tiles = {}

        def wd1_fetch(c):
            wt = wring.tile([128, DCH * 512], F16, tag="wd1c",
                            name=f"wd1c{c}")
            nc.sync.dma_start(
                out=wt.rearrange("p (k c) -> p k c", k=DCH, c=512),
                in_=wd1_d[c * DCH * 128:(c + 1) * DCH * 128, :].rearrange(
                    "(k p) c -> p k c", p=128))
            wd1_tiles[c] = wt

        def zpre_psum_step(layer, t, src, wx, bc):
            # x-projection of step t straight into its own PSUM bank;
            # the recurrent gate matmuls later accumulate on top.
            zp = pp.tile([128, 128], F32, tag="z", bufs=3,
                         name=f"z{layer}_{t}")
            zs_tiles[(layer, t)] = zp
            # start=True marks the whole 2KB PSUM zero-region pending-
            # zero, so only the FIRST matmul of the tile may set it;
            # later writes to pending bytes overwrite (not accumulate).
            for g in range(4):
                mm(zp[:, g * B:(g + 1) * B],
                   wx[:, g * 128:(g + 1) * 128],
                   src[:, t * B:(t + 1) * B],
                   start=(g == 0), stop=False, skip_group_check=True)
            if LSTM_BIAS:
                for g in range(4):
                    nc.vector.tensor_scalar(
                        zp[:, g * B:(g + 1) * B],
                        zp[:, g * B:(g + 1) * B],
                        bc[:, g:g + 1], None, op0=ALU.add)

        def dense_step(t):
            wt = wd1_tiles[t // DCH]
            kk = t % DCH
            for m in range(4):
                mm(dacc4[:, m * B:(m + 1) * B],
                   wt[:, kk * 512 + m * 128: kk * 512 + (m + 1) * 128],
                   out2[:, t * B:(t + 1) * B],
                   start=(t == 0 and m == 0),
                   stop=(t == T - 1 and m == 3),
                   skip_group_check=True)

        def lstm_scan(layer, wh, wx, bc, src, outbuf, m_pool, tc_pool,
                      extra=None, extra_end=None):
            # extra(t) is emitted between step t's matmuls and its
            # elementwise ops; extra_end(t) after the full step. Either
            # may only emit work whose data deps are on steps <= t-2
            # (so the in-order PE stream never stalls).
            for t in range(T):
                s = layer * T + t
                cur = SB0 if s % 2 == 0 else SB1
                prev = SB1 if s % 2 == 0 else SB0
                if s == 0:
                    h_prev = hz
                elif t == 0:
                    h_prev = out1[:, (T - 1) * B: T * B]
                else:
                    h_prev = outbuf[:, (t - 1) * B: t * B]

                # keep the zpre stream two steps ahead of consumption
                if t + 2 < T:
                    zpre_psum_step(layer, t + 2, src, wx, bc)

                zp = zs_tiles[(layer, t)]
                for g in range(4):
                    mm(zp[:, g * B:(g + 1) * B],
                       wh[:, g * 128:(g + 1) * 128],
                       h_prev, start=False, stop=(g == 3),
                       skip_group_check=True)

                if extra is not None:
                    extra(t)

                # one fused sigmoid over all 4 gates; g-gate preact was
                # pre-scaled 2x on host so tanh(zg) = 2*sig(2*zg)-1
                nc.scalar.activation(cur[:, 0:128], zp, AF.Sigmoid)
                nc.vector.tensor_scalar(prev[:, 160:192], cur[:, 96:128],
                                        2.0, -1.0, op0=ALU.mult,
                                        op1=ALU.add)
                M = m_pool.tile([128, 64], F16, tag="M")
                nc.vector.tensor_tensor(M, cur[:, 0:64], prev[:, 128:192],
                                        op=ALU.mult)
                nc.vector.tensor_tensor(cur[:, 128:160], M[:, 0:B],
                                        M[:, B:2 * B], op=ALU.add)
                TC = tc_pool.tile([128, B], F16, tag="TC")
                nc.scalar.activation(TC, cur[:, 128:160], AF.Tanh)
                nc.vector.tensor_tensor(outbuf[:, t * B:(t + 1) * B],
                                        cur[:, 64:96], TC, op=ALU.mult)
                if extra_end is not None:
                    extra_end(t)
            if extra is not None:
                for t in range(T, T + 2):
                    extra(t)

        def scan1_end(t):
            # feed the conv2/conv3 stream; drain at ~1 thunk per step
            # (bursts delay the next step's recurrent matmuls)
            if t % 16 == 2:
                n = t // 16
                if n + 3 < NCH:
                    q_conv2(n + 3)
                if n + 2 < NCH:
                    q_conv3(n + 2)
            if t == 200:
                wd1_fetch(0)
                wd1_fetch(1)
            for _ in range(2 if (t < 32 or t % 4 == 0) else 1):
                if conv_q:
                    conv_q.pop(0)()

        zpre_psum_step(0, 0, y3, wx1, b1c)
        zpre_psum_step(0, 1, y3, wx1, b1c)
        with tc.tile_pool(name="mp1", bufs=3) as mp1, \
                tc.tile_pool(name="tcp", bufs=3) as tcp:
            lstm_scan(0, wh1, wx1, b1c, y3, out1, mp1, tcp,
                      extra_end=scan1_end)
        while conv_q:
            conv_q.pop(0)()
        cvpA.__exit__(None, None, None)
        cvpB.__exit__(None, None, None)

        def scan2_extra(t):
            td = t - 2
            if td >= 0:
                dense_step(td)
                if td % DCH == DCH - 1 and td // DCH + 2 < T // DCH:
                    wd1_fetch(td // DCH + 2)

        zpre_psum_step(1, 0, out1, wx2, b2c)
        zpre_psum_step(1, 1, out1, wx2, b2c)
        with tc.tile_pool(name="mp2", bufs=3) as mp2, \
                tc.tile_pool(name="tcp2", bufs=3) as tcp2:
            lstm_scan(1, wh2, wx2, b2c, out1, out2, mp2, tcp2,
                      extra=scan2_extra)

        # ---------------- BN, dense2, softmax ----------------
        with tc.tile_pool(name="fin", bufs=1) as fin:
            dsb = [fin.tile([128, B], F32, name=f"dsb{m}") for m in range(4)]
            sq = fin.tile([128, B], F32, tag="sqt", bufs=2)
            stats = fin.tile([128, 8], F32)
            for m in range(4):
                nc.scalar.activation(dsb[m], dacc4[:, m * B:(m + 1) * B],
                                     AF.Identity, bias=bd1c[:, m:m + 1])
                nc.vector.tensor_reduce(stats[:, m:m + 1], dsb[m], axis=AX.X,
                                        op=ALU.add)
                nc.scalar.activation(sq, dsb[m], AF.Square)
                nc.vector.tensor_reduce(stats[:, 4 + m:5 + m], sq, axis=AX.X,
                                        op=ALU.add)

            nc.gpsimd.dma_start(out=cc_in[:, :], in_=stats)
            # AllGather + local reduce is ~2x cheaper than AllReduce here
            nc.gpsimd.collective_compute(
                "AllGather", ALU.bypass,
                replica_groups=[list(range(NCORES))],
                ins=[cc_in[:, :]], outs=[cc_out[:, :]])
            statsa = fin.tile([128, NCORES * 8], F32)
            nc.gpsimd.dma_start(
                out=statsa.rearrange("p (c s) -> p c s", c=NCORES, s=8),
                in_=cc_out[:, :].rearrange("(c p) s -> p c s", p=128))
            statsg = fin.tile([128, 8], F32)
            nc.vector.tensor_reduce(
                statsg.rearrange("p (s o) -> p s o", s=8, o=1),
                statsa.rearrange("p (c s) -> p s c", c=NCORES, s=8),
                axis=AX.X, op=ALU.add)

            meanv = fin.tile([128, 4], F32)
            nc.vector.tensor_scalar(meanv, statsg[:, 0:4], 1.0 / 256.0, None,
                                    op0=ALU.mult)
            ex2 = fin.tile([128, 4], F32)
            nc.vector.tensor_scalar(ex2, statsg[:, 4:8], 1.0 / 256.0, None,
                                    op0=ALU.mult)
            msq = fin.tile([128, 4], F32)
            nc.vector.tensor_tensor(msq, meanv, meanv, op=ALU.mult)
            varv = fin.tile([128, 4], F32)
            nc.vector.tensor_tensor(varv, ex2, msq, op=ALU.subtract)
            vpe = fin.tile([128, 4], F32)
            nc.vector.tensor_scalar(vpe, varv, EPS, None, op0=ALU.add)
            rec = fin.tile([128, 4], F32)
            nc.vector.reciprocal(rec, vpe)
            rstd = fin.tile([128, 4], F32)
            nc.scalar.activation(rstd, rec, AF.Sqrt)
            av = fin.tile([128, 4], F32)
            nc.vector.tensor_tensor(av, rstd, bng, op=ALU.mult)
            mb = fin.tile([128, 4], F32)
            nc.vector.tensor_tensor(mb, meanv, av, op=ALU.mult)
            bv = fin.tile([128, 4], F32)
            nc.vector.tensor_tensor(bv, bnb, mb, op=ALU.subtract)

            o2 = pp.tile([B, NB], F32, tag="o2", bufs=1)
            for m in range(4):
                tmp = fin.tile([128, B], F32, tag="tmp", bufs=2)
                nc.vector.tensor_scalar(tmp, dsb[m], av[:, m:m + 1],
                                        bv[:, m:m + 1], op0=ALU.mult,
                                        op1=ALU.add)
                tmp2 = fin.tile([128, B], F32, tag="tmp2", bufs=2)
                nc.vector.tensor_scalar(tmp2, tmp, SLOPE, None, op0=ALU.mult)
                dbn = fin.tile([128, B], F16, tag="dbn", bufs=4)
                nc.vector.tensor_tensor(dbn, tmp, tmp2, op=ALU.max)
                mm(o2, dbn, wd2[:, m * NB:(m + 1) * NB],
                   start=(m == 0), stop=(m == 3))

            sm = fin.tile([B, NB], F32)
            nc.vector.tensor_tensor(sm, o2, bd2r, op=ALU.add)
            mx = fin.tile([B, 1], F32)
            nc.vector.tensor_reduce(mx, sm, axis=AX.X, op=ALU.max)
            xs = fin.tile([B, NB], F32)
            nc.vector.tensor_scalar(xs, sm, mx, None, op0=ALU.subtract)
            ex = fin.tile([B, NB], F32)
            sume = fin.tile([B, 1], F32)
            nc.scalar.activation(ex, xs, AF.Exp)
            nc.vector.tensor_reduce(sume, ex, axis=AX.X, op=ALU.add)
            rcs = fin.tile([B, 1], F32)
            nc.vector.reciprocal(rcs, sume)
            res = fin.tile([B, NB], F32)
            nc.vector.tensor_scalar(res, ex, rcs, None, op0=ALU.mult)
            nc.gpsimd.dma_start(out=out_d[:, :], in_=res)
            if _DBG_DUMP:
                nc.gpsimd.dma_start(out=dbg_y3_d[:, :], in_=y3)
                nc.gpsimd.dma_start(out=dbg_o1_d[:, :], in_=out1)
                nc.gpsimd.dma_start(out=dbg_o2_d[:, :], in_=out2)
                for m in range(4):
                    nc.gpsimd.dma_start(
                        out=dbg_da_d[:, m * B:(m + 1) * B], in_=dsb[m])

    if _SPLIT_WAITS:
        _split_waits(nc, keep=_SPLIT_KEEP)
    return nc


_SEQ_ONLY = ("InstEventSemaphore",)
_SPLIT_WAITS = True
_SPLIT_KEEP = 1


def _split_waits(nc, keep=1):
    """Walrus engine-instruction structs hold very few sync-wait commands.
    Hoist all but `keep` waits of every engine instruction into standalone
    single-wait EventSemaphore sequencer instructions placed just before it
    (same engine stream, so ordering is preserved)."""
    uid = [0]
    for fn in nc.m.functions:
        for bb in fn.blocks:
            insts = bb.instructions
            out = []
            changed = False
            for ins in insts:
                si = ins.sync_info
                tn = type(ins).__name__
                if (si is not None and tn not in _SEQ_ONLY
                        and len(si.on_wait) > keep):
                    waits = list(si.on_wait)
                    for w in waits[:-keep] if keep else waits:
                        uid[0] += 1
                        ev = mybir.InstEventSemaphore(
                            name=f"xw_{uid[0]}_{ins.name}",
                            engine=ins.engine,
                            ins=[], outs=[],
                            sync_info=mybir.SyncInfo(on_wait=[w], on_update=[]),
                        )
                        out.append(ev)
                    ins.sync_info = mybir.SyncInfo(
                        on_wait=waits[-keep:] if keep else [],
                        on_update=list(si.on_update))
                    changed = True
                out.append(ins)
            if changed:
                bb.instructions = out
    return nc


_PROGRAMS = {}


def _prepare_inputs(inputs):
    x = _f32(inputs["x"])
    convW1 = _f32(inputs["convW1"])
    convW2 = _f32(inputs["convW2"])
    convW3 = _f32(inputs["convW3"])
    for nm in ("convb1", "convb2", "convb3"):
        assert np.abs(np.asarray(inputs[nm])).max() == 0.0, "conv bias unsupported"

    w1 = np.concatenate([convW1[k] for k in range(5)], axis=1)
    w2 = convW2.reshape(5 * 32, 512)
    w2a, w2b = w2[0:128], w2[128:160]
    w3 = np.concatenate([convW3[k, kt * 128:(kt + 1) * 128, :]
                         for k in range(5) for kt in range(4)], axis=1)

    wx1 = _perm_gates(_f32(inputs["Wx1"]))
    wh1 = _perm_gates(_f32(inputs["Wh1"]))
    wx2 = _perm_gates(_f32(inputs["Wx2"]))
    wh2 = _perm_gates(_f32(inputs["Wh2"]))
    b1 = _perm_gates(_f32(inputs["b1"])[None, :])[0]
    b2 = _perm_gates(_f32(inputs["b2"])[None, :])[0]
    b1c = b1.reshape(4, 128).T.copy()
    b2c = b2.reshape(4, 128).T.copy()

    wd1 = _f32(inputs["Wd1"])
    bd1c = _f32(inputs["bd1"]).reshape(4, 128).T.copy()
    bng = _f32(inputs["bn_scale"]).reshape(4, 128).T.copy()
    bnb = _f32(inputs["bn_bias"]).reshape(4, 128).T.copy()
    wd2 = _f32(inputs["Wd2"])
    wd2c = np.concatenate([wd2[m * 128:(m + 1) * 128, :] for m in range(4)],
                          axis=1)
    bd2r = np.tile(_f32(inputs["bd2"])[None, :], (B, 1))

    shared = dict(
        w1=_h(w1), w2a=_h(w2a), w2b=_h(w2b), w3=_h(w3),
        wx1=_h(wx1), wh1=_h(wh1), wx2=_h(wx2), wh2=_h(wh2),
        b1c=b1c, b2c=b2c,
        wd1=_h(wd1), bd1c=bd1c, bng=bng, bnb=bnb,
        wd2=_h(wd2c), bd2r=bd2r,
    )

    in_maps = []
    for c in range(NCORES):
        xs = x[c * B:(c + 1) * B]
        xT = xs.transpose(2, 1, 0).reshape(CIN, F)
        xTp = np.zeros((CIN, PF), np.float32)
        xTp[:, PAD:PAD + F] = xT
        m = dict(shared)
        m["xT"] = _h(xTp)
        in_maps.append(m)
    return in_maps


def kernel(**inputs) -> np.ndarray:
    lstm_bias = bool(np.abs(np.asarray(inputs["b1"])).max() > 0
                     or np.abs(np.asarray(inputs["b2"])).max() > 0)
    if lstm_bias not in _PROGRAMS:
        _PROGRAMS[lstm_bias] = build_program(lstm_bias)
    in_maps = _prepare_inputs(inputs)
    res = run_bass_kernel_spmd(_PROGRAMS[lstm_bias], in_maps,
                               list(range(NCORES)))
    outs = [res.results[c]["out"] for c in range(NCORES)]
    return np.concatenate(outs, axis=0).astype(np.float32)


if __name__ == "__main__":
    import reference
    ins = {k: np.asarray(v) for k, v in reference.setup_inputs().items()}
    got = kernel(**ins)
    print(got.shape, got.dtype, got[:2])



# revision 5
# speedup vs baseline: 1.0220x; 1.0220x over previous
"""Trainium2 Bass kernel for nn_AutoLSTM: conv1d x3 -> LSTM x2 -> dense+BN -> softmax.

Data-parallel over batch: 8 cores x 32 rows, weights replicated.
Layout: free dim is (t, b) time-major with b inner (32), padded by 2 time
steps of zeros each side for the SAME convs.  LSTM runs transposed:
[H=128 partitions, 32 batch] tiles.

Schedule: the 512-step recurrence is latency-bound (~1.2us/step chain:
PE gate matmuls -> fused all-gate sigmoid on ACT -> DVE c-update -> ACT
tanh -> DVE h), so everything else hides inside it:
 - x-projections (zpre) are matmul'd JIT into per-step PSUM banks two
   steps ahead; the recurrent matmuls accumulate on top (only the first
   matmul of a bank may set start=True: start marks the whole 2KB PSUM
   zero-region pending-zero and later writes there OVERWRITE).
 - conv2/conv3 chunks stream into scan1 as a paced thunk queue (~1/step);
   conv activations use Prelu, which shares the ACT function table with
   Sigmoid/Tanh (Lrelu does not -> 1.3us table reload per switch).
 - the im2col for conv2 is built by Pool (GPSIMD) copies; Pool cannot
   access PSUM.
 - dense1 accumulates per-step during scan2 from a DMA ring of Wd1
   chunks; BN stats cross-core reduction is AllGather + local reduce
   (cheaper than AllReduce in latency).

All elementwise scan state is fp16 (DVE 2x mode); rel err ~3.5e-3.

Walrus constraint: engine instructions hold few sync-waits, so
_split_waits hoists extras into standalone EventSemaphore sequencer ops.
"""

import sys

for p in ("/opt/trn_rl_repo",):
    if p not in sys.path:
        sys.path.insert(0, p)

from contextlib import ExitStack

import numpy as np

import concourse.bass as bass
import concourse.mybir as mybir
from concourse.tile import TileContext
from concourse.bass_utils import run_bass_kernel_spmd

F32 = mybir.dt.float32
F16 = mybir.dt.float16
AF = mybir.ActivationFunctionType
ALU = mybir.AluOpType
AX = mybir.AxisListType

NCORES = 8
B = 32          # per-core batch
T = 256
CIN = 8
H = 128
NB = 10
EPS = 1e-5
SLOPE = 0.01    # jax.nn.leaky_relu default

PT = T + 4              # padded time
PF = PT * B             # 8320
F = T * B               # 8192
PAD = 2 * B             # 64
NCH = 16                # 512-wide (16t x 32b) chunks
CH = 512
DCH = 8                 # dense: t-tiles per streamed Wd1 ring chunk
ZQ = 8                  # LSTM steps per PSUM zpre chunk

# on-chip gate order [f, i, g, o]; jax order is [i, f, g, o].  g before o
# so the fused sigmoid covers only [f,i,g] (96 cols) — the o-gate gets its
# own off-critical-path sigmoid (the h op needs it ~500ns later).
GATE_PERM = [1, 0, 2, 3]


def _h(x):
    return np.asarray(x, dtype=np.float16)


def _f32(x):
    return np.ascontiguousarray(np.asarray(x, dtype=np.float32))


def _perm_gates(w):
    # on-chip order [f, i, o, g]; g-gate preactivation scaled 2x so the
    # fused all-sigmoid gate op yields tanh(zg) = 2*sigmoid(2*zg) - 1
    blocks = [w[..., s * H:(s + 1) * H] for s in GATE_PERM]
    blocks[2] = blocks[2] * 2.0
    return np.concatenate(blocks, axis=-1)


_DBG_RELU = False       # CoreSim lacks Lrelu; swap in Relu for sim debugging
_DBG_DUMP = False       # add intermediate DRAM dumps for debugging


def build_program(lstm_bias=False):
    LSTM_BIAS = lstm_bias
    # Prelu (parametric_relu) == leaky relu with alpha, and it lives in the
    # same activation table set as Sigmoid/Tanh/Identity/Square — no 1.3us
    # table reloads when conv activations interleave with the LSTM scan.
    AF_L = AF.Relu if _DBG_RELU else AF.Prelu
    nc = bass.Bass()

    P = nc.declare_dram_parameter
    xT_d = P("xT", [CIN, PF], F16, isOutput=False)
    w1_d = P("w1", [CIN, 5 * 32], F16, isOutput=False)
    w2a_d = P("w2a", [128, 512], F16, isOutput=False)
    w2b_d = P("w2b", [32, 512], F16, isOutput=False)
    w3_d = P("w3", [128, 20 * 128], F16, isOutput=False)
    wx1_d = P("wx1", [128, 512], F16, isOutput=False)
    wh1_d = P("wh1", [128, 512], F16, isOutput=False)
    wx2_d = P("wx2", [128, 512], F16, isOutput=False)
    wh2_d = P("wh2", [128, 512], F16, isOutput=False)
    b1_d = P("b1c", [128, 4], F32, isOutput=False)
    b2_d = P("b2c", [128, 4], F32, isOutput=False)
    wd1_d = P("wd1", [T * H, 512], F16, isOutput=False)
    bd1_d = P("bd1c", [128, 4], F32, isOutput=False)
    bng_d = P("bng", [128, 4], F32, isOutput=False)
    bnb_d = P("bnb", [128, 4], F32, isOutput=False)
    wd2_d = P("wd2", [128, 4 * NB], F16, isOutput=False)
    bd2_d = P("bd2r", [B, NB], F32, isOutput=False)
    out_d = P("out", [B, NB], F32, isOutput=True)
    if _DBG_DUMP:
        dbg_y3_d = P("dbg_y3", [128, F], F16, isOutput=True)
        dbg_o1_d = P("dbg_o1", [128, F], F16, isOutput=True)
        dbg_o2_d = P("dbg_o2", [128, F], F16, isOutput=True)
        dbg_da_d = P("dbg_da", [128, 4 * B], F32, isOutput=True)

    cc_in = nc.dram_tensor("cc_in", [128, 8], F32)
    cc_out = nc.dram_tensor("cc_out", [NCORES * 128, 8], F32)

    with TileContext(nc) as tc, ExitStack() as ctx:
        mm = lambda *a, **k: nc.tensor.matmul(*a, **k)

        wp = ctx.enter_context(tc.tile_pool(name="wp", bufs=1))
        mp = ctx.enter_context(tc.tile_pool(name="mp", bufs=1))
        pp = ctx.enter_context(tc.tile_pool(name="psum", bufs=1, space="PSUM"))
        wring = ctx.enter_context(tc.tile_pool(name="wring", bufs=2))

        # persistent activation storages
        out1 = mp.tile([128, F], F16)
        out2 = mp.tile([128, F], F16)
        y3 = mp.tile([128, F], F16)
        # scan state ping-pong: cols 0:128 = sigmoid(z) [sf si so sg],
        # 128:160 = c, 160:192 = w = 2*sg-1 (tanh of g-gate preactivation)
        SB0 = mp.tile([128, 192], F16)
        SB1 = mp.tile([128, 192], F16)
        hz = mp.tile([128, B], F16)
        nc.vector.memset(hz, 0.0)
        nc.vector.memset(SB1[:, 128:160], 0.0)

        # conv working buffers: xT dies after conv1; y1/y2 live through
        # scan1 (conv2/conv3 are streamed into it)
        cvpB = tc.tile_pool(name="cvpB", bufs=1)
        cvpA = tc.tile_pool(name="cvpA", bufs=1)
        cpB = cvpB.__enter__()
        cpA = cvpA.__enter__()

        y1 = cpA.tile([32, PF], F16)
        im2 = cpA.tile([128, PF], F16)
        y2 = [cpB.tile([128, PF], F16, name=f"y2_{m}", tag=f"y2_{m}")
              for m in range(4)]

        # x input DMA first — it gates conv1
        cvpX = tc.tile_pool(name="cvpX", bufs=1)
        cpX = cvpX.__enter__()
        xT = cpX.tile([CIN, PF], F16)
        nc.sync.dma_start(out=xT, in_=xT_d[:, :])

        # ---- stage all weights through DVE so matmul operands and ACT
        # bias operands have single-sem producers ----
        with tc.tile_pool(name="stg", bufs=1) as stg:
            def wload(shape, dram, nm, dt=F16, dst=None):
                raw = stg.tile(shape, dt, tag=f"r_{nm}", name=f"r_{nm}")
                nc.sync.dma_start(out=raw, in_=dram[:, :])
                t = dst if dst is not None else wp.tile(shape, dt, name=nm,
                                                        tag=nm)
                nc.vector.tensor_copy(t, raw)
                return t

            w1 = wload([CIN, 5 * 32], w1_d, "w1f")
            w2a = wload([128, 512], w2a_d, "w2af")
            w2b = wload([32, 512], w2b_d, "w2bf")
            w3 = wload([128, 20 * 128], w3_d, "w3f")
            wx1 = wload([128, 512], wx1_d, "wx1f")
            wh1 = wload([128, 512], wh1_d, "wh1f")
            wx2 = wload([128, 512], wx2_d, "wx2f")
            wh2 = wload([128, 512], wh2_d, "wh2f")
            wd2 = wload([128, 4 * NB], wd2_d, "wd2f")
            b1c = wload([128, 4], b1_d, "b1f", F32)
            b2c = wload([128, 4], b2_d, "b2f", F32)
            bd1c = wload([128, 4], bd1_d, "bd1f", F32)
            bng = wload([128, 4], bng_d, "bngf", F32)
            bnb = wload([128, 4], bnb_d, "bnbf", F32)
            bd2r = wload([B, NB], bd2_d, "bd2f", F32)

        # ---------------- conv stack ----------------
        nc.scalar.memzero(y1[:, 0:PAD])
        nc.scalar.memzero(y1[:, PF - PAD:PF])
        for m in range(4):
            nc.scalar.memzero(y2[m][:, 0:PAD])
            nc.scalar.memzero(y2[m][:, PF - PAD:PF])

        # conv1: 5 taps, K=8 -> 32 ch (fully pre-scan)
        for n in range(NCH):
            ps = pp.tile([32, CH], F32, tag="big", bufs=3)
            for k in range(5):
                mm(ps, w1[:, k * 32:(k + 1) * 32],
                   xT[:, n * CH + k * B: n * CH + k * B + CH],
                   start=(k == 0), stop=(k == 4))
            nc.scalar.activation(y1[:, PAD + n * CH: PAD + (n + 1) * CH], ps,
                                 AF_L, alpha=SLOPE)
        cvpX.__exit__(None, None, None)

        # conv2 im2col (taps 0..3) built by Pool copies in PF/4 column
        # quarters — Pool is idle during the scan, and quarter granularity
        # lets early conv2 chunks start before the whole im2col is done.
        nc.gpsimd.memset(im2[0:32, 0:2 * B], 0.0)
        nc.gpsimd.memset(im2[32:64, 0:B], 0.0)
        nc.gpsimd.memset(im2[96:128, PF - B:PF], 0.0)
        QW = PF // 4
        for q in range(4):
            for j in range(4):
                sh = (j - 2) * B
                d0, d1 = q * QW, (q + 1) * QW
                s0, s1 = max(0, d0 + sh), min(PF, d1 + sh)
                if s1 > s0:
                    nc.gpsimd.tensor_copy(
                        im2[j * 32:(j + 1) * 32, s0 - sh:s1 - sh],
                        y1[:, s0:s1])

        # conv2/conv3 are emitted chunk-wise: a prefix pre-scan, the rest
        # drained as small thunks inside scan1 (engines are mostly idle
        # during the latency-bound recurrence).
        def emit_conv2(n):
            for m in range(4):
                ps = pp.tile([128, CH], F32, tag="big", bufs=3)
                mm(ps, w2a[:, m * 128:(m + 1) * 128],
                   im2[:, PAD + n * CH: PAD + (n + 1) * CH],
                   start=True, stop=False)
                mm(ps, w2b[:, m * 128:(m + 1) * 128],
                   y1[:, PAD + n * CH + 2 * B: PAD + n * CH + 2 * B + CH],
                   start=False, stop=True)
                nc.scalar.activation(
                    y2[m][:, PAD + n * CH: PAD + (n + 1) * CH],
                    ps, AF_L, alpha=SLOPE)

        def conv3_mms(ps, n, lo, hi):
            for idx in range(lo, hi):
                k, kt = idx // 4, idx % 4
                mm(ps, w3[:, (k * 4 + kt) * 128:(k * 4 + kt + 1) * 128],
                   y2[kt][:, n * CH + k * B: n * CH + k * B + CH],
                   start=(idx == 0), stop=(idx == 19), skip_group_check=True)

        def emit_conv3(n):
            ps = pp.tile([128, CH], F32, tag="big", bufs=3)
            conv3_mms(ps, n, 0, 20)
            nc.scalar.activation(y3[:, n * CH:(n + 1) * CH], ps,
                                 AF_L, alpha=SLOPE)

        conv_q = []

        def q_conv2(n):
            for m in range(4):
                def mms(n=n, m=m):
                    ps = pp.tile([128, CH], F32, tag="big", bufs=3,
                                 name=f"c2_{n}_{m}")
                    conv_q_state[(n, m)] = ps
                    mm(ps, w2a[:, m * 128:(m + 1) * 128],
                       im2[:, PAD + n * CH: PAD + (n + 1) * CH],
                       start=True, stop=False)
                    mm(ps, w2b[:, m * 128:(m + 1) * 128],
                       y1[:, PAD + n * CH + 2 * B:
                          PAD + n * CH + 2 * B + CH],
                       start=False, stop=True)

                def act(n=n, m=m):
                    ps = conv_q_state.pop((n, m))
                    nc.scalar.activation(
                        y2[m][:, PAD + n * CH: PAD + (n + 1) * CH],
                        ps, AF_L, alpha=SLOPE)
                conv_q.append(mms)
                conv_q.append(act)

        def q_conv3(n):
            for lo in range(0, 20, 3):
                def mms(n=n, lo=lo):
                    if lo == 0:
                        conv_q_state[("c3", n)] = pp.tile(
                            [128, CH], F32, tag="big", bufs=3,
                            name=f"c3_{n}")
                    conv3_mms(conv_q_state[("c3", n)], n, lo,
                              min(lo + 3, 20))
                conv_q.append(mms)

            def act(n=n):
                ps = conv_q_state.pop(("c3", n))
                nc.scalar.activation(y3[:, n * CH:(n + 1) * CH], ps,
                                     AF_L, alpha=SLOPE)
            conv_q.append(act)

        conv_q_state = {}

        # pre-scan prefix: enough conv2/conv3 for the scan's first chunks
        # minimum prefix that gates scan step 0 (zpre(0,0/1) needs y3
        # chunk 0 only); conv2(2)/conv3(1) go to the thunk queue head and
        # drain at 2/step over the first scan steps
        for n in range(2):
            emit_conv2(n)
        emit_conv3(0)
        q_conv2(2)
        q_conv3(1)

        # ---------------- LSTM phase ----------
        # x-projections (zpre) are matmul'd straight into PSUM chunks of
        # ZQ steps (ping-pong); the recurrent gate matmuls accumulate on
        # top and the fused sigmoid reads PSUM directly.
        dacc4 = pp.tile([128, 4 * B], F32, name="dacc4", tag="dacc",
                        bufs=1)
        wd1_tiles = {}
        zs_tiles = {}

        def wd1_fetch(c):
            wt = wring.tile([128, DCH * 512], F16, tag="wd1c",
                            name=f"wd1c{c}")
            nc.sync.dma_start(
                out=wt.rearrange("p (k c) -> p k c", k=DCH, c=512),
                in_=wd1_d[c * DCH * 128:(c + 1) * DCH * 128, :].rearrange(
                    "(k p) c -> p k c", p=128))
            wd1_tiles[c] = wt

        def zpre_psum_step(layer, t, src, wx, bc):
            # x-projection of step t straight into its own PSUM bank;
            # the recurrent gate matmuls later accumulate on top.
            zp = pp.tile([128, 128], F32, tag="z", bufs=3,
                         name=f"z{layer}_{t}")
            zs_tiles[(layer, t)] = zp
            # start=True marks the whole 2KB PSUM zero-region pending-
            # zero, so only the FIRST matmul of the tile may set it;
            # later writes to pending bytes overwrite (not accumulate).
            for g in range(4):
                mm(zp[:, g * B:(g + 1) * B],
                   wx[:, g * 128:(g + 1) * 128],
                   src[:, t * B:(t + 1) * B],
                   start=(g == 0), stop=False, skip_group_check=True)
            if LSTM_BIAS:
                for g in range(4):
                    nc.vector.tensor_scalar(
                        zp[:, g * B:(g + 1) * B],
                        zp[:, g * B:(g + 1) * B],
                        bc[:, g:g + 1], None, op0=ALU.add)

        def dense_step(t):
            wt = wd1_tiles[t // DCH]
            kk = t % DCH
            for m in range(4):
                mm(dacc4[:, m * B:(m + 1) * B],
                   wt[:, kk * 512 + m * 128: kk * 512 + (m + 1) * 128],
                   out2[:, t * B:(t + 1) * B],
                   start=(t == 0 and m == 0),
                   stop=(t == T - 1 and m == 3),
                   skip_group_check=True)

        def lstm_scan(layer, wh, wx, bc, src, outbuf, m_pool, tc_pool,
                      extra=None, extra_end=None):
            # extra(t) is emitted between step t's matmuls and its
            # elementwise ops; extra_end(t) after the full step. Either
            # may only emit work whose data deps are on steps <= t-2
            # (so the in-order PE stream never stalls).
            for t in range(T):
                s = layer * T + t
                cur = SB0 if s % 2 == 0 else SB1
                prev = SB1 if s % 2 == 0 else SB0
                if s == 0:
                    h_prev = hz
                elif t == 0:
                    h_prev = out1[:, (T - 1) * B: T * B]
                else:
                    h_prev = outbuf[:, (t - 1) * B: t * B]

                # keep the zpre stream two steps ahead of consumption
                if t + 2 < T:
                    zpre_psum_step(layer, t + 2, src, wx, bc)

                zp = zs_tiles[(layer, t)]
                for g in range(4):
                    mm(zp[:, g * B:(g + 1) * B],
                       wh[:, g * 128:(g + 1) * 128],
                       h_prev, start=False, stop=(g == 3),
                       skip_group_check=True)

                if extra is not None:
                    extra(t)

                # fused sigmoid over [f,i,g] only (96 cols) — the w/M/add
                # chain needs just these; o-gate sigmoid runs as a second
                # ACT op in the engine-idle window before tanh (h consumes
                # it much later).  g-gate preact was pre-scaled 2x on host
                # so tanh(zg) = 2*sig(2*zg)-1.
                nc.scalar.activation(cur[:, 0:96], zp[:, 0:96], AF.Sigmoid)
                nc.scalar.activation(cur[:, 96:128], zp[:, 96:128],
                                     AF.Sigmoid)
                nc.vector.tensor_scalar(prev[:, 160:192], cur[:, 64:96],
                                        2.0, -1.0, op0=ALU.mult,
                                        op1=ALU.add)
                M = m_pool.tile([128, 64], F16, tag="M")
                nc.vector.tensor_tensor(M, cur[:, 0:64], prev[:, 128:192],
                                        op=ALU.mult)
                nc.vector.tensor_tensor(cur[:, 128:160], M[:, 0:B],
                                        M[:, B:2 * B], op=ALU.add)
                TC = tc_pool.tile([128, B], F16, tag="TC")
                nc.scalar.activation(TC, cur[:, 128:160], AF.Tanh)
                nc.vector.tensor_tensor(outbuf[:, t * B:(t + 1) * B],
                                        cur[:, 96:128], TC, op=ALU.mult)
                if extra_end is not None:
                    extra_end(t)
            if extra is not None:
                for t in range(T, T + 2):
                    extra(t)

        def scan1_end(t):
            # feed the conv2/conv3 stream; drain at ~1 thunk per step
            # (bursts delay the next step's recurrent matmuls)
            if t % 16 == 2:
                n = t // 16
                if n + 3 < NCH:
                    q_conv2(n + 3)
                if n + 2 < NCH:
                    q_conv3(n + 2)
            if t == 200:
                wd1_fetch(0)
                wd1_fetch(1)
            for _ in range(2 if (t < 32 or t % 4 == 0) else 1):
                if conv_q:
                    conv_q.pop(0)()

        zpre_psum_step(0, 0, y3, wx1, b1c)
        zpre_psum_step(0, 1, y3, wx1, b1c)
        with tc.tile_pool(name="mp1", bufs=3) as mp1, \
                tc.tile_pool(name="tcp", bufs=3) as tcp:
            lstm_scan(0, wh1, wx1, b1c, y3, out1, mp1, tcp,
                      extra_end=scan1_end)
        while conv_q:
            conv_q.pop(0)()
        cvpA.__exit__(None, None, None)
        cvpB.__exit__(None, None, None)

        def scan2_extra(t):
            td = t - 2
            if td >= 0:
                dense_step(td)
                if td % DCH == DCH - 1 and td // DCH + 2 < T // DCH:
                    wd1_fetch(td // DCH + 2)

        zpre_psum_step(1, 0, out1, wx2, b2c)
        zpre_psum_step(1, 1, out1, wx2, b2c)
        with tc.tile_pool(name="mp2", bufs=3) as mp2, \
                tc.tile_pool(name="tcp2", bufs=3) as tcp2:
            lstm_scan(1, wh2, wx2, b2c, out1, out2, mp2, tcp2,
                      extra=scan2_extra)

        # ---------------- BN, dense2, softmax ----------------
        with tc.tile_pool(name="fin", bufs=1) as fin:
            dsb = [fin.tile([128, B], F32, name=f"dsb{m}") for m in range(4)]
            sq = fin.tile([128, B], F32, tag="sqt", bufs=2)
            stats = fin.tile([128, 8], F32)
            for m in range(4):
                nc.scalar.activation(dsb[m], dacc4[:, m * B:(m + 1) * B],
                                     AF.Identity, bias=bd1c[:, m:m + 1])
                nc.vector.tensor_reduce(stats[:, m:m + 1], dsb[m], axis=AX.X,
                                        op=ALU.add)
                nc.scalar.activation(sq, dsb[m], AF.Square)
                nc.vector.tensor_reduce(stats[:, 4 + m:5 + m], sq, axis=AX.X,
                                        op=ALU.add)

            nc.gpsimd.dma_start(out=cc_in[:, :], in_=stats)
            # AllGather + local reduce is ~2x cheaper than AllReduce here
            nc.gpsimd.collective_compute(
                "AllGather", ALU.bypass,
                replica_groups=[list(range(NCORES))],
                ins=[cc_in[:, :]], outs=[cc_out[:, :]])
            statsa = fin.tile([128, NCORES * 8], F32)
            nc.gpsimd.dma_start(
                out=statsa.rearrange("p (c s) -> p c s", c=NCORES, s=8),
                in_=cc_out[:, :].rearrange("(c p) s -> p c s", p=128))
            statsg = fin.tile([128, 8], F32)
            nc.vector.tensor_reduce(
                statsg.rearrange("p (s o) -> p s o", s=8, o=1),
                statsa.rearrange("p (c s) -> p s c", c=NCORES, s=8),
                axis=AX.X, op=ALU.add)

            meanv = fin.tile([128, 4], F32)
            nc.vector.tensor_scalar(meanv, statsg[:, 0:4], 1.0 / 256.0, None,
                                    op0=ALU.mult)
            ex2 = fin.tile([128, 4], F32)
            nc.vector.tensor_scalar(ex2, statsg[:, 4:8], 1.0 / 256.0, None,
                                    op0=ALU.mult)
            msq = fin.tile([128, 4], F32)
            nc.vector.tensor_tensor(msq, meanv, meanv, op=ALU.mult)
            varv = fin.tile([128, 4], F32)
            nc.vector.tensor_tensor(varv, ex2, msq, op=ALU.subtract)
            vpe = fin.tile([128, 4], F32)
            nc.vector.tensor_scalar(vpe, varv, EPS, None, op0=ALU.add)
            rec = fin.tile([128, 4], F32)
            nc.vector.reciprocal(rec, vpe)
            rstd = fin.tile([128, 4], F32)
            nc.scalar.activation(rstd, rec, AF.Sqrt)
            av = fin.tile([128, 4], F32)
            nc.vector.tensor_tensor(av, rstd, bng, op=ALU.mult)
            mb = fin.tile([128, 4], F32)
            nc.vector.tensor_tensor(mb, meanv, av, op=ALU.mult)
            bv = fin.tile([128, 4], F32)
            nc.vector.tensor_tensor(bv, bnb, mb, op=ALU.subtract)

            o2 = pp.tile([B, NB], F32, tag="o2", bufs=1)
            for m in range(4):
                tmp = fin.tile([128, B], F32, tag="tmp", bufs=2)
                nc.vector.tensor_scalar(tmp, dsb[m], av[:, m:m + 1],
                                        bv[:, m:m + 1], op0=ALU.mult,
                                        op1=ALU.add)
                tmp2 = fin.tile([128, B], F32, tag="tmp2", bufs=2)
                nc.vector.tensor_scalar(tmp2, tmp, SLOPE, None, op0=ALU.mult)
                dbn = fin.tile([128, B], F16, tag="dbn", bufs=4)
                nc.vector.tensor_tensor(dbn, tmp, tmp2, op=ALU.max)
                mm(o2, dbn, wd2[:, m * NB:(m + 1) * NB],
                   start=(m == 0), stop=(m == 3))

            sm = fin.tile([B, NB], F32)
            nc.vector.tensor_tensor(sm, o2, bd2r, op=ALU.add)
            mx = fin.tile([B, 1], F32)
            nc.vector.tensor_reduce(mx, sm, axis=AX.X, op=ALU.max)
            xs = fin.tile([B, NB], F32)
            nc.vector.tensor_scalar(xs, sm, mx, None, op0=ALU.subtract)
            ex = fin.tile([B, NB], F32)
            sume = fin.tile([B, 1], F32)
            nc.scalar.activation(ex, xs, AF.Exp)
            nc.vector.tensor_reduce(sume, ex, axis=AX.X, op=ALU.add)
            rcs = fin.tile([B, 1], F32)
            nc.vector.reciprocal(rcs, sume)
            res = fin.tile([B, NB], F32)
            nc.vector.tensor_scalar(res, ex, rcs, None, op0=ALU.mult)
            nc.gpsimd.dma_start(out=out_d[:, :], in_=res)
            if _DBG_DUMP:
                nc.gpsimd.dma_start(out=dbg_y3_d[:, :], in_=y3)
                nc.gpsimd.dma_start(out=dbg_o1_d[:, :], in_=out1)
                nc.gpsimd.dma_start(out=dbg_o2_d[:, :], in_=out2)
                for m in range(4):
                    nc.gpsimd.dma_start(
                        out=dbg_da_d[:, m * B:(m + 1) * B], in_=dsb[m])

    if _SPLIT_WAITS:
        _split_waits(nc, keep=_SPLIT_KEEP)
    return nc


_SEQ_ONLY = ("InstEventSemaphore",)
_SPLIT_WAITS = True
_SPLIT_KEEP = 1


def _split_waits(nc, keep=1):
    """Walrus engine-instruction structs hold very few sync-wait commands.
    Hoist all but `keep` waits of every engine instruction into standalone
    single-wait EventSemaphore sequencer instructions placed just before it
    (same engine stream, so ordering is preserved)."""
    uid = [0]
    for fn in nc.m.functions:
        for bb in fn.blocks:
            insts = bb.instructions
            out = []
            changed = False
            for ins in insts:
                si = ins.sync_info
                tn = type(ins).__name__
                if (si is not None and tn not in _SEQ_ONLY
                        and len(si.on_wait) > keep):
                    waits = list(si.on_wait)
                    for w in waits[:-keep] if keep else waits:
                        uid[0] += 1
                        ev = mybir.InstEventSemaphore(
                            name=f"xw_{uid[0]}_{ins.name}",
                            engine=ins.engine,
                            ins=[], outs=[],
                            sync_info=mybir.SyncInfo(on_wait=[w], on_update=[]),
                        )
                        out.append(ev)
                    ins.sync_info = mybir.SyncInfo(
                        on_wait=waits[-keep:] if keep else [],
                        on_update=list(si.on_update))
                    changed = True
                out.append(ins)
            if changed:
                bb.instructions = out
    return nc


_PROGRAMS = {}


def _prepare_inputs(inputs):
    x = _f32(inputs["x"])
    convW1 = _f32(inputs["convW1"])
    convW2 = _f32(inputs["convW2"])
    convW3 = _f32(inputs["convW3"])
    for nm in ("convb1", "convb2", "convb3"):
        assert np.abs(np.asarray(inputs[nm])).max() == 0.0, "conv bias unsupported"

    w1 = np.concatenate([convW1[k] for k in range(5)], axis=1)
    w2 = convW2.reshape(5 * 32, 512)
    w2a, w2b = w2[0:128], w2[128:160]
    w3 = np.concatenate([convW3[k, kt * 128:(kt + 1) * 128, :]
                         for k in range(5) for kt in range(4)], axis=1)

    wx1 = _perm_gates(_f32(inputs["Wx1"]))
    wh1 = _perm_gates(_f32(inputs["Wh1"]))
    wx2 = _perm_gates(_f32(inputs["Wx2"]))
    wh2 = _perm_gates(_f32(inputs["Wh2"]))
    b1 = _perm_gates(_f32(inputs["b1"])[None, :])[0]
    b2 = _perm_gates(_f32(inputs["b2"])[None, :])[0]
    b1c = b1.reshape(4, 128).T.copy()
    b2c = b2.reshape(4, 128).T.copy()

    wd1 = _f32(inputs["Wd1"])
    bd1c = _f32(inputs["bd1"]).reshape(4, 128).T.copy()
    bng = _f32(inputs["bn_scale"]).reshape(4, 128).T.copy()
    bnb = _f32(inputs["bn_bias"]).reshape(4, 128).T.copy()
    wd2 = _f32(inputs["Wd2"])
    wd2c = np.concatenate([wd2[m * 128:(m + 1) * 128, :] for m in range(4)],
                          axis=1)
    bd2r = np.tile(_f32(inputs["bd2"])[None, :], (B, 1))

    shared = dict(
        w1=_h(w1), w2a=_h(w2a), w2b=_h(w2b), w3=_h(w3),
        wx1=_h(wx1), wh1=_h(wh1), wx2=_h(wx2), wh2=_h(wh2),
        b1c=b1c, b2c=b2c,
        wd1=_h(wd1), bd1c=bd1c, bng=bng, bnb=bnb,
        wd2=_h(wd2c), bd2r=bd2r,
    )

    in_maps = []
    for c in range(NCORES):
        xs = x[c * B:(c + 1) * B]
        xT = xs.transpose(2, 1, 0).reshape(CIN, F)
        xTp = np.zeros((CIN, PF), np.float32)
        xTp[:, PAD:PAD + F] = xT
        m = dict(shared)
        m["xT"] = _h(xTp)
        in_maps.append(m)
    return in_maps


def kernel(**inputs) -> np.ndarray:
    lstm_bias = bool(np.abs(np.asarray(inputs["b1"])).max() > 0
                     or np.abs(np.asarray(inputs["b2"])).max() > 0)
    if lstm_bias not in _PROGRAMS:
        _PROGRAMS[lstm_bias] = build_program(lstm_bias)
    in_maps = _prepare_inputs(inputs)
    res = run_bass_kernel_spmd(_PROGRAMS[lstm_bias], in_maps,
                               list(range(NCORES)))
    outs = [res.results[c]["out"] for c in range(NCORES)]
    return np.concatenate(outs, axis=0).astype(np.float32)


if __name__ == "__main__":
    import reference
    ins = {k: np.asarray(v) for k, v in reference.setup_inputs().items()}
    got = kernel(**ins)
    print(got.shape, got.dtype, got[:2])

